# revision 18
# baseline (speedup 1.0000x reference)
"""Block-diagonal linear layer (16 blocks of 256x256) on 8 TRN2 NeuronCores.

Sharding: expert-style over num_blocks — each core owns 2 of the 16 blocks
(a 512-wide feature slice of x and y) for the full 16384-row batch. The
TensorEngine contracts over the partition dim, so x is pre-transposed on the
host to feature-major layout ([4096, 16384]); core c's shard is then the
contiguous row slice [c*512:(c+1)*512] (no per-core copies). The kernel
computes yT[o, n] = sum_i W[k, o, i] * xT[k*256+i, n] + b[k, o] for its two
blocks and the host transposes the gathered output back. Compared to
batch-data-parallel this moves the same x/y bytes but only 1/8th of the
weights per core, and the host-side reshard is a single transpose.

The kernel is memory-bound (~34MB of DMA traffic per core, HBM shared per
core-pair), so x/W/y are carried as bf16 on the wire (PSUM accumulation
stays f32); measured rel err vs the f32 reference is ~2.5e-3.
"""

import sys

import numpy as np

try:
    import concourse  # noqa: F401
except ImportError:
    sys.path.insert(0, "/opt/trn_rl_repo")

NUM_BLOCKS = 16
IN_FEATURES = 4096
OUT_FEATURES = 4096
BLOCK_IN = 256
BLOCK_OUT = 256
BATCH = 16384
NCORES = 8
BLOCKS_PER_CORE = NUM_BLOCKS // NCORES  # 2
FEAT = BLOCKS_PER_CORE * BLOCK_IN  # 512 features per core
NCHUNK = 2048  # batch columns per SBUF tile

# "bf16": x/W/y bf16 on the wire, f32 PSUM accumulate (fast, rel err ~2.5e-3)
# "f32r": everything f32, matmul in float32r mode (rel err ~1e-4)
MODE = "bf16"

# test.py toggles these for profiling.
TRACE = False
TRACE_CORES = None
LAST_EXEC_NS = None
LAST_RESULT = None

_BUILT = {}


def _build(mode: str):
    """Build + compile the single-core Bass program (identical SPMD on 8 cores)."""
    import concourse.mybir as mybir
    import concourse.tile as tile
    from concourse import bacc

    nc = bacc.Bacc("TRN2", target_bir_lowering=False, debug=False)
    f32 = mybir.dt.float32
    mm_dt = mybir.dt.bfloat16 if mode == "bf16" else mybir.dt.float32r
    out_dt = mybir.dt.bfloat16 if mode == "bf16" else f32

    ncc = FEAT // 128  # feature chunks per core (4)
    nblks = BATCH // NCHUNK  # 8
    # x/y are host-packed so every [128, NCHUNK] tile is one contiguous
    # 512KB block: row-block (fc*nblks + nblk) holds feature-chunk fc,
    # batch-chunk nblk.
    xT = nc.dram_tensor("xT", [ncc * nblks * 128, NCHUNK], mm_dt, kind="ExternalInput").ap()
    Wh = nc.dram_tensor("Wh", [128, ncc * 256], mm_dt, kind="ExternalInput").ap()
    bh = nc.dram_tensor("bh", [128, ncc], f32, kind="ExternalInput").ap()
    yT = nc.dram_tensor("yT", [ncc * nblks * 128, NCHUNK], out_dt, kind="ExternalOutput").ap()

    NFREE = 512  # one fp32 PSUM bank
    n4s = NCHUNK // NFREE  # 4

    with tile.TileContext(nc) as tc:
        with (
            tc.tile_pool(name="wp", bufs=1) as wpool,
            tc.tile_pool(name="xp", bufs=16) as xpool,
            tc.tile_pool(name="yp", bufs=6) as ypool,
            tc.tile_pool(name="pp", bufs=8, space="PSUM") as ppool,
        ):
            # Weights + bias ride the SWDGE (gpsimd) ring so the x loads on
            # the SP HWDGE ring start streaming at t=0 in parallel.
            w_all = wpool.tile([128, ncc * 256], mm_dt)
            nc.gpsimd.dma_start(out=w_all[:], in_=Wh[:])
            bias_sb = wpool.tile([128, ncc], f32)
            nc.gpsimd.dma_start(out=bias_sb[:], in_=bh[:])

            for nblk in range(nblks):
                xt = {}
                for kl in range(BLOCKS_PER_CORE):
                    for i2 in range(2):
                        t = xpool.tile([128, NCHUNK], mm_dt, tag="xt")
                        r0 = ((kl * 2 + i2) * nblks + nblk) * 128
                        nc.sync.dma_start(out=t[:], in_=xT[r0 : r0 + 128, :])
                        xt[kl, i2] = t
                for kl in range(BLOCKS_PER_CORE):
                    for o2 in range(2):
                        c = kl * 2 + o2
                        y_sb = ypool.tile([128, NCHUNK], out_dt, tag="yt")
                        for n4 in range(n4s):
                            ps = ppool.tile([128, NFREE], f32)
                            for i2 in range(2):
                                w0 = (kl * 2 + i2) * 256 + o2 * 128
                                nc.tensor.matmul(
                                    ps[:],
                                    lhsT=w_all[:, w0 : w0 + 128],
                                    rhs=xt[kl, i2][:, n4 * NFREE : (n4 + 1) * NFREE],
                                    start=(i2 == 0),
                                    stop=(i2 == 1),
                                )
                            # PSUM evacuation + bias add, split across ACT
                            # and DVE so neither engine becomes the wall.
                            y_slice = y_sb[:, n4 * NFREE : (n4 + 1) * NFREE]
                            if n4 % 2 == 0:
                                nc.scalar.activation(
                                    y_slice,
                                    ps[:],
                                    mybir.ActivationFunctionType.Identity,
                                    bias=bias_sb[:, c : c + 1],
                                )
                            else:
                                nc.vector.tensor_scalar_add(
                                    y_slice, ps[:], bias_sb[:, c : c + 1]
                                )
                        # y stores alternate between the ACT HWDGE ring and
                        # the SWDGE ring; keeping them off the SP ring avoids
                        # head-of-line-blocking the x loads.
                        store_eng = nc.scalar if c % 2 == 0 else nc.gpsimd
                        s0 = (c * nblks + nblk) * 128
                        store_eng.dma_start(
                            out=yT[s0 : s0 + 128, :], in_=y_sb[:]
                        )

    nc.compile()
    return nc


def _get_nc(mode: str):
    if mode not in _BUILT:
        _BUILT[mode] = _build(mode)
    return _BUILT[mode]


def kernel(x: np.ndarray, W: np.ndarray, b: np.ndarray) -> np.ndarray:
    global LAST_EXEC_NS, LAST_RESULT
    from concourse.bass_utils import run_bass_kernel_spmd

    assert x.shape == (BATCH, IN_FEATURES) and x.dtype == np.float32
    nc = _get_nc(MODE)

    if MODE == "bf16":
        import ml_dtypes

        wire_dt = np.dtype(ml_dtypes.bfloat16)
    else:
        wire_dt = np.dtype(np.float32)

    # Pack per-core x images: row-block (fc*nblks+nblk) of core c is the
    # contiguous (feature-major) tile of features [c*512+fc*128, +128) x
    # batch rows [nblk*2048, +2048). Single transpose+cast pass.
    ncc = FEAT // 128
    nblks = BATCH // NCHUNK
    xTp = (
        x.reshape(nblks, NCHUNK, NCORES, ncc, 128)
        .transpose(2, 3, 0, 4, 1)  # [c, fc, nblk, p, nn]
        .astype(wire_dt)
        .reshape(NCORES, ncc * nblks * 128, NCHUNK)
    )
    # Weight image per core: Wh[p, (kl*2+i2)*256 + o] = W[c*2+kl, o, i2*128+p]
    Whs = (
        W.transpose(0, 2, 1)  # [k, i, o]
        .reshape(NCORES, BLOCKS_PER_CORE * 2, 128, BLOCK_OUT)  # [c, kl*2+i2, p, o]
        .transpose(0, 2, 1, 3)  # [c, p, ci, o]
        .reshape(NCORES, 128, BLOCKS_PER_CORE * 2 * BLOCK_OUT)
    ).astype(wire_dt)
    # Bias image per core: bh[p, kl*2+o2] = b[c*2+kl, o2*128+p]
    bhs = (
        b.reshape(NCORES, BLOCKS_PER_CORE * 2, 128)
        .transpose(0, 2, 1)
        .astype(np.float32)
    )
    bhs = np.ascontiguousarray(bhs)

    in_maps = [
        {
            "xT": xTp[c],
            "Wh": np.ascontiguousarray(Whs[c]),
            "bh": bhs[c],
        }
        for c in range(NCORES)
    ]

    res = run_bass_kernel_spmd(
        nc, in_maps, list(range(NCORES)), trace=TRACE, trace_cores=TRACE_CORES
    )
    LAST_EXEC_NS = res.exec_time_ns
    LAST_RESULT = res

    # Unpack: shard row-block (cc*nblks+nblk) holds y features
    # [c*512+cc*128, +128) x batch rows [nblk*2048, +2048), feature-major.
    ys = np.stack([res.results[c]["yT"] for c in range(NCORES)])
    y = (
        ys.reshape(NCORES, ncc, nblks, 128, NCHUNK)
        .transpose(2, 4, 0, 1, 3)  # [nblk, nn, c, cc, p]
        .astype(np.float32)
        .reshape(BATCH, OUT_FEATURES)
    )
    return y


# revision 22
# speedup vs baseline: 1.1264x; 1.1264x over previous
"""Block-diagonal linear layer (16 blocks of 256x256) on 8 TRN2 NeuronCores.

Sharding: expert-style over num_blocks — each core owns 2 of the 16 blocks
(a 512-wide feature slice of x and y) for the full 16384-row batch. The
TensorEngine contracts over the partition dim, so x is pre-transposed on the
host to feature-major layout ([4096, 16384]); core c's shard is then the
contiguous row slice [c*512:(c+1)*512] (no per-core copies). The kernel
computes yT[o, n] = sum_i W[k, o, i] * xT[k*256+i, n] + b[k, o] for its two
blocks and the host transposes the gathered output back. Compared to
batch-data-parallel this moves the same x/y bytes but only 1/8th of the
weights per core, and the host-side reshard is a single transpose.

The kernel is memory-bound (~34MB of DMA traffic per core, HBM shared per
core-pair), so x/W/y are carried as bf16 on the wire (PSUM accumulation
stays f32); measured rel err vs the f32 reference is ~2.5e-3.
"""

import sys

import numpy as np

try:
    import concourse  # noqa: F401
except ImportError:
    sys.path.insert(0, "/opt/trn_rl_repo")

NUM_BLOCKS = 16
IN_FEATURES = 4096
OUT_FEATURES = 4096
BLOCK_IN = 256
BLOCK_OUT = 256
BATCH = 16384
NCORES = 8
BLOCKS_PER_CORE = NUM_BLOCKS // NCORES  # 2
FEAT = BLOCKS_PER_CORE * BLOCK_IN  # 512 features per core
NCHUNK = 2048  # batch columns per SBUF tile

# "f16": x/W/y float16 on the wire, f32 PSUM accumulate (fast, rel err ~3e-4)
# "bf16": same traffic/speed as f16 but 7-bit mantissa (rel err ~2.5e-3)
# "f32r": everything f32, matmul in float32r mode (rel err ~1e-4, ~2.2x slower)
MODE = "f16"

# test.py toggles these for profiling.
TRACE = False
TRACE_CORES = None
LAST_EXEC_NS = None
LAST_RESULT = None

_BUILT = {}


def _build(mode: str):
    """Build + compile the single-core Bass program (identical SPMD on 8 cores)."""
    import concourse.mybir as mybir
    import concourse.tile as tile
    from concourse import bacc

    nc = bacc.Bacc("TRN2", target_bir_lowering=False, debug=False)
    f32 = mybir.dt.float32
    wire = {"f16": mybir.dt.float16, "bf16": mybir.dt.bfloat16}
    mm_dt = wire.get(mode, mybir.dt.float32r)
    out_dt = wire.get(mode, f32)

    ncc = FEAT // 128  # feature chunks per core (4)
    nblks = BATCH // NCHUNK  # 8
    # x/y are host-packed so every [128, NCHUNK] tile is one contiguous
    # 512KB block: row-block (fc*nblks + nblk) holds feature-chunk fc,
    # batch-chunk nblk.
    xT = nc.dram_tensor("xT", [ncc * nblks * 128, NCHUNK], mm_dt, kind="ExternalInput").ap()
    Wh = nc.dram_tensor("Wh", [128, ncc * 256], mm_dt, kind="ExternalInput").ap()
    bh = nc.dram_tensor("bh", [128, ncc], f32, kind="ExternalInput").ap()
    yT = nc.dram_tensor("yT", [ncc * nblks * 128, NCHUNK], out_dt, kind="ExternalOutput").ap()

    NFREE = 512  # one fp32 PSUM bank
    n4s = NCHUNK // NFREE  # 4

    with tile.TileContext(nc) as tc:
        with (
            tc.tile_pool(name="wp", bufs=1) as wpool,
            tc.tile_pool(name="xp", bufs=16) as xpool,
            tc.tile_pool(name="yp", bufs=6) as ypool,
            tc.tile_pool(name="pp", bufs=8, space="PSUM") as ppool,
        ):
            # Weights + bias ride the SWDGE (gpsimd) ring so the x loads on
            # the SP HWDGE ring start streaming at t=0 in parallel.
            w_all = wpool.tile([128, ncc * 256], mm_dt)
            nc.scalar.dma_start(out=w_all[:], in_=Wh[:])
            bias_sb = wpool.tile([128, ncc], f32)
            nc.scalar.dma_start(out=bias_sb[:], in_=bh[:])

            for nblk in range(nblks):
                xt = {}
                for kl in range(BLOCKS_PER_CORE):
                    for i2 in range(2):
                        t = xpool.tile([128, NCHUNK], mm_dt, tag="xt")
                        r0 = ((kl * 2 + i2) * nblks + nblk) * 128
                        nc.sync.dma_start(out=t[:], in_=xT[r0 : r0 + 128, :])
                        xt[kl, i2] = t
                for kl in range(BLOCKS_PER_CORE):
                    for o2 in range(2):
                        c = kl * 2 + o2
                        y_sb = ypool.tile([128, NCHUNK], out_dt, tag="yt")
                        for n4 in range(n4s):
                            ps = ppool.tile([128, NFREE], f32)
                            for i2 in range(2):
                                w0 = (kl * 2 + i2) * 256 + o2 * 128
                                nc.tensor.matmul(
                                    ps[:],
                                    lhsT=w_all[:, w0 : w0 + 128],
                                    rhs=xt[kl, i2][:, n4 * NFREE : (n4 + 1) * NFREE],
                                    start=(i2 == 0),
                                    stop=(i2 == 1),
                                )
                            # PSUM evacuation + bias add, split across ACT
                            # and DVE so neither engine becomes the wall.
                            y_slice = y_sb[:, n4 * NFREE : (n4 + 1) * NFREE]
                            if n4 % 2 == 0:
                                nc.scalar.activation(
                                    y_slice,
                                    ps[:],
                                    mybir.ActivationFunctionType.Identity,
                                    bias=bias_sb[:, c : c + 1],
                                )
                            else:
                                nc.vector.tensor_scalar_add(
                                    y_slice, ps[:], bias_sb[:, c : c + 1]
                                )
                        # y stores alternate between the ACT HWDGE ring and
                        # the SWDGE ring; keeping them off the SP ring avoids
                        # head-of-line-blocking the x loads. The final batch
                        # chunk stores in halves to shorten the kernel tail.
                        store_eng = nc.scalar if c % 2 == 0 else nc.gpsimd
                        s0 = (c * nblks + nblk) * 128
                        if nblk == nblks - 1:
                            half = NCHUNK // 2
                            store_eng.dma_start(
                                out=yT[s0 : s0 + 128, :half], in_=y_sb[:, :half]
                            )
                            store_eng.dma_start(
                                out=yT[s0 : s0 + 128, half:], in_=y_sb[:, half:]
                            )
                        else:
                            store_eng.dma_start(
                                out=yT[s0 : s0 + 128, :], in_=y_sb[:]
                            )

    nc.compile()
    return nc


def _get_nc(mode: str):
    if mode not in _BUILT:
        _BUILT[mode] = _build(mode)
    return _BUILT[mode]


def kernel(x: np.ndarray, W: np.ndarray, b: np.ndarray) -> np.ndarray:
    global LAST_EXEC_NS, LAST_RESULT
    from concourse.bass_utils import run_bass_kernel_spmd

    assert x.shape == (BATCH, IN_FEATURES) and x.dtype == np.float32
    nc = _get_nc(MODE)

    if MODE == "f16":
        wire_dt = np.dtype(np.float16)
    elif MODE == "bf16":
        import ml_dtypes

        wire_dt = np.dtype(ml_dtypes.bfloat16)
    else:
        wire_dt = np.dtype(np.float32)

    # Pack per-core x images: row-block (fc*nblks+nblk) of core c is the
    # contiguous (feature-major) tile of features [c*512+fc*128, +128) x
    # batch rows [nblk*2048, +2048). Single transpose+cast pass.
    ncc = FEAT // 128
    nblks = BATCH // NCHUNK
    xTp = (
        x.reshape(nblks, NCHUNK, NCORES, ncc, 128)
        .transpose(2, 3, 0, 4, 1)  # [c, fc, nblk, p, nn]
        .astype(wire_dt)
        .reshape(NCORES, ncc * nblks * 128, NCHUNK)
    )
    # Weight image per core: Wh[p, (kl*2+i2)*256 + o] = W[c*2+kl, o, i2*128+p]
    Whs = (
        W.transpose(0, 2, 1)  # [k, i, o]
        .reshape(NCORES, BLOCKS_PER_CORE * 2, 128, BLOCK_OUT)  # [c, kl*2+i2, p, o]
        .transpose(0, 2, 1, 3)  # [c, p, ci, o]
        .reshape(NCORES, 128, BLOCKS_PER_CORE * 2 * BLOCK_OUT)
    ).astype(wire_dt)
    # Bias image per core: bh[p, kl*2+o2] = b[c*2+kl, o2*128+p]
    bhs = (
        b.reshape(NCORES, BLOCKS_PER_CORE * 2, 128)
        .transpose(0, 2, 1)
        .astype(np.float32)
    )
    bhs = np.ascontiguousarray(bhs)

    in_maps = [
        {
            "xT": xTp[c],
            "Wh": np.ascontiguousarray(Whs[c]),
            "bh": bhs[c],
        }
        for c in range(NCORES)
    ]

    # Transient NRT/device hiccups (e.g. NRT_EXEC_UNIT_UNRECOVERABLE) have
    # been observed on this fleet and clear after a short wait; retry a few
    # times before giving up.
    import time

    last_err = None
    for attempt in range(4):
        try:
            res = run_bass_kernel_spmd(
                nc, in_maps, list(range(NCORES)), trace=TRACE, trace_cores=TRACE_CORES
            )
            break
        except Exception as e:  # noqa: BLE001
            last_err = e
            time.sleep(10 * (attempt + 1))
    else:
        raise last_err
    LAST_EXEC_NS = res.exec_time_ns
    LAST_RESULT = res

    # Unpack: shard row-block (cc*nblks+nblk) holds y features
    # [c*512+cc*128, +128) x batch rows [nblk*2048, +2048), feature-major.
    ys = np.stack([res.results[c]["yT"] for c in range(NCORES)])
    y = (
        ys.reshape(NCORES, ncc, nblks, 128, NCHUNK)
        .transpose(2, 4, 0, 1, 3)  # [nblk, nn, c, cc, p]
        .astype(np.float32)
        .reshape(BATCH, OUT_FEATURES)
    )
    return y
